# revision 22
# baseline (speedup 1.0000x reference)
"""Multi-head attention (RoPE + softmax) forward for Trainium2, 8 NeuronCores.

Problem: B=4, S=2048, D=2048, H=16 heads (hd=128), fp32 in/out.

Sharding: core c handles batch b = c//2 and head-group g = c%2 (8 heads).
The two partial output projections per batch are summed on the host.

v4 design (vs v2 baseline, 1.66-2.13ms HW; v4 measures ~0.95-1.2ms, sim
span 684us with PE 97% busy = the bf16 streaming roofline):
  - The softmax denominator no longer uses N=1 transposed matmuls (4096 of
    them; each needs a fresh 128-col LDWEIGHTS that the sim models at 0ns
    but HW pays ~110ns+ for).  Instead: bf16 chunked-sequential adds over
    the 16 e-tiles on the (underused) DVE (4 groups of 4, in-place, then a
    root merge - bounded rounding like a depth-4 tree), then one
    gpsimd partition_all_reduce per (head, q-chunk) does the partition
    sum AND the broadcast on the otherwise-idle Pool engine.
  - exp runs on [128, 1024] tiles (2 PSUM banks) - halves ACT instruction
    count and the 352-cycle per-instruction overhead.
  - pso is drained to attn_all immediately via raw ACT copies;
    normalization (in-place DVE reciprocal of the all-reduced denominator,
    then an in-place DVE multiply) trails the PE, off the critical path.
  - v projection widened to N=512 rhs (LDWEIGHTS now hides under the
    longer streams); v/wo PSUM->SBUF copies moved to the idle DVE.
  - ident / ones_p transpose-broadcast path deleted.
  - output written bf16 (halves the 16MB out-DMA; host sums in fp32) and
    wo quarter 0 is preloaded during attention, trimming the stage-3 tail.
  - fp8/DoubleRow was evaluated and rejected: numpy ablation on the real
    inputs gives 6.4e-2 rel err with fp8 projections (gate is 2e-2).
"""

import math

import numpy as np

B, S, D = 4, 2048, 2048
H_PER_CORE = 8  # heads per core
HD = 128  # head dim
F = 1024  # features per core (head group)
P = 128
DT = 16  # contraction tiles over D
NCORES = 8
SCALE = 1.0 / math.sqrt(HD)

_CACHE = {}


def _build():
    import concourse.bacc as bacc
    import concourse.bass_isa as bass_isa
    import concourse.mybir as mybir
    import concourse.tile as tile

    f32 = mybir.dt.float32
    bf16 = mybir.dt.bfloat16
    EXP = mybir.ActivationFunctionType.Exp

    nc = bacc.Bacc("TRN2", target_bir_lowering=False, debug=False, num_devices=NCORES)

    xT = nc.dram_tensor("xT", [P, 4, DT, 512], bf16, kind="ExternalInput")
    wq = nc.dram_tensor("wq", [P, H_PER_CORE, DT, P], bf16, kind="ExternalInput")
    wk = nc.dram_tensor("wk", [P, H_PER_CORE, DT, P], bf16, kind="ExternalInput")
    wv = nc.dram_tensor("wv", [P, 2, DT, 512], bf16, kind="ExternalInput")
    wo = nc.dram_tensor("wo", [P, 4, H_PER_CORE, 512], bf16, kind="ExternalInput")
    cosT_d = nc.dram_tensor("cosT", [P, S], bf16, kind="ExternalInput")
    sinT_d = nc.dram_tensor("sinT", [P, S], bf16, kind="ExternalInput")
    mask_d = nc.dram_tensor("maskT", [P, DT], f32, kind="ExternalInput")
    out_d = nc.dram_tensor("out", [S, D], bf16, kind="ExternalOutput")

    with tile.TileContext(nc) as tc, nc.allow_low_precision(
        reason="bf16 matmul tiles; PSUM accumulation stays fp32"
    ):
        with (
            tc.tile_pool(name="const", bufs=1) as constp,
            tc.tile_pool(name="big", bufs=1) as bigp,
            tc.tile_pool(name="cs", bufs=1) as csp,
            tc.tile_pool(name="wp", bufs=2) as wp,
            tc.tile_pool(name="wo0p", bufs=1) as wo0p,
        ):
            mask_sb = constp.tile([P, DT], f32)
            wo0_sb = wo0p.tile([P, H_PER_CORE, 512], bf16, name="wo0")

            v_all = bigp.tile([P, DT, F], bf16, name="v_all")
            attn_all = bigp.tile([P, H_PER_CORE, S], bf16, name="attn_all")
            cos_sb = csp.tile([P, S], bf16)
            sin_sb = csp.tile([P, S], bf16)

            with (
                tc.tile_pool(name="xp", bufs=1) as xp,
                tc.tile_pool(name="pss", bufs=2, space="PSUM") as pssP,
                tc.tile_pool(name="pso", bufs=2, space="PSUM") as psoP,
                tc.tile_pool(name="ps1", bufs=2, space="PSUM") as ps1P,
            ):
                x_all = xp.tile([P, DT, S], bf16)

                def x_quarter(sc):
                    nc.sync.dma_start(
                        out=x_all[:, :, sc * 512 : (sc + 1) * 512], in_=xT[:, sc]
                    )

                # ---- stage 1: v projection (x streams in behind it) ----
                with tc.tile_pool(name="wvp", bufs=2) as wvp:
                    wv_t = [None, None]
                    wv_t[0] = wvp.tile([P, DT, 512], bf16, tag="wv", name="wv0")
                    # fine-grained first chunks: the st=0 chain's dt=0 matmul
                    # can start after ~320KB instead of ~1.5MB of DMA
                    nc.sync.dma_start(out=wv_t[0][:, 0:2], in_=wv[:, 0, 0:2])
                    nc.sync.dma_start(
                        out=x_all[:, 0:2, 0:256], in_=xT[:, 0, 0:2, 0:256]
                    )
                    nc.sync.dma_start(out=wv_t[0][:, 2:8], in_=wv[:, 0, 2:8])
                    nc.sync.dma_start(
                        out=x_all[:, 2:8, 0:256], in_=xT[:, 0, 2:8, 0:256]
                    )
                    nc.sync.dma_start(out=wv_t[0][:, 8:DT], in_=wv[:, 0, 8:DT])
                    nc.sync.dma_start(
                        out=x_all[:, 8:DT, 0:256], in_=xT[:, 0, 8:DT, 0:256]
                    )
                    nc.sync.dma_start(out=mask_sb[:], in_=mask_d[:])
                    for fh in range(2):
                        wvt = wv_t[fh]
                        for st in range(16):
                            if fh == 0 and st == 0:
                                nc.sync.dma_start(
                                    out=x_all[:, :, 256:512], in_=xT[:, 0, :, 256:512]
                                )
                            if fh == 0 and st in (2, 3, 4):
                                x_quarter(st - 1)
                            if fh == 0 and st == 10:
                                nc.sync.dma_start(out=cos_sb[:], in_=cosT_d[:])
                                nc.sync.dma_start(out=sin_sb[:], in_=sinT_d[:])
                            if fh == 0 and st == 5:
                                nwv = wvp.tile([P, DT, 512], bf16, tag="wv", name="wv1")
                                nc.sync.dma_start(out=nwv[:], in_=wv[:, 1])
                                wv_t[1] = nwv
                            pool = (ps1P, psoP)[st % 2]
                            ps = pool.tile(
                                [P, 512], f32,
                                tag="ps1" if pool is ps1P else "pso",
                                name="ps_v",
                            )
                            for dt in range(DT):
                                nc.tensor.matmul(
                                    ps[:],
                                    lhsT=x_all[:, dt, st * P : (st + 1) * P],
                                    rhs=wvt[:, dt],
                                    start=(dt == 0),
                                    stop=(dt == DT - 1),
                                )
                            nc.vector.tensor_copy(
                                v_all[:, st, fh * 512 : (fh + 1) * 512], ps[:]
                            )

                # ---- stage 2: merged qk projection + attention pipeline ----
                with (
                    tc.tile_pool(name="qkp", bufs=2) as qkp,
                    tc.tile_pool(name="ep", bufs=3) as ep,
                    tc.tile_pool(name="trp", bufs=2) as trp,
                    tc.tile_pool(name="rootp", bufs=1) as rootp,
                    tc.tile_pool(name="tp", bufs=2) as tp,
                    tc.tile_pool(name="bp", bufs=2) as bp,
                ):
                    w_t = {}

                    def emit_w(h):
                        for wd, tag in ((wq, "wq"), (wk, "wk")):
                            wt = wp.tile([P, DT, P], bf16, tag=tag)
                            nc.sync.dma_start(out=wt[:], in_=wd[:, h])
                            w_t[(tag, h)] = wt

                    qk_t = {}

                    def proj_chunks(h):
                        for tag, dtag in (("wq", "q"), ("wk", "k")):
                            wt = w_t.pop((tag, h))
                            dst = qkp.tile([P, S], bf16, tag=dtag)
                            qk_t[(dtag, h)] = dst
                            for sc in range(4):
                                ps = ps1P.tile([P, 512], f32, tag="ps1", name="ps_proj")
                                for dt in range(DT):
                                    nc.tensor.matmul(
                                        ps[:],
                                        lhsT=wt[:, dt],
                                        rhs=x_all[:, dt, sc * 512 : (sc + 1) * 512],
                                        start=(dt == 0),
                                        stop=(dt == DT - 1),
                                    )
                                t2 = tp.tile([P, 512], f32, tag="t2")
                                cs = cos_sb[:, sc * 512 : (sc + 1) * 512]
                                sn = sin_sb[:, sc * 512 : (sc + 1) * 512]
                                dsl = dst[:, sc * 512 : (sc + 1) * 512]
                                nc.vector.tensor_mul(t2[0:64, :], ps[64:128, :], sn[0:64, :])
                                nc.vector.tensor_mul(t2[64:128, :], ps[0:64, :], sn[64:128, :])
                                nc.vector.tensor_mul(dsl, ps[:], cs)
                                nc.vector.tensor_add(dsl, dsl, t2[:])
                                yield

                    def attn_head(h, filler):
                        q_sb = qk_t.pop(("q", h))
                        k_sb = qk_t.pop(("k", h))
                        for qc2 in range(2):
                            q0 = qc2 * 1024
                            pso0 = psoP.tile([P, 512], f32, tag="pso", name="pso0")
                            pso1 = psoP.tile([P, 512], f32, tag="pso", name="pso1")
                            pss_t = {}

                            def scores(kt):
                                ps = pssP.tile([P, 1024], f32, tag="pss")
                                for half in range(2):
                                    nc.tensor.matmul(
                                        ps[:, half * 512 : (half + 1) * 512],
                                        lhsT=k_sb[:, kt * P : (kt + 1) * P],
                                        rhs=q_sb[
                                            :,
                                            q0 + half * 512 : q0 + (half + 1) * 512,
                                        ],
                                        start=True,
                                        stop=True,
                                        skip_group_check=True,
                                    )
                                pss_t[kt] = ps

                            # denominator accumulation: 4 groups of 4 e-tiles
                            # summed sequentially (in-place), then merged into
                            # a root tile.  bf16 adds run at 2x DVE rate.
                            acc_state = {"acc": None, "root": None}

                            def tree_push(e, kt):
                                gpos = kt % 4
                                if gpos == 0:
                                    acc_state["e_first"] = e
                                elif gpos == 1:
                                    acc = trp.tile([P, 1024], bf16, tag="acc4")
                                    nc.vector.tensor_add(
                                        acc[:], acc_state.pop("e_first")[:], e[:]
                                    )
                                    acc_state["acc"] = acc
                                else:
                                    acc = acc_state["acc"]
                                    nc.vector.tensor_add(acc[:], acc[:], e[:])
                                    if gpos == 3:
                                        root = acc_state["root"]
                                        if kt == 3:
                                            pass  # first group: becomes root base
                                        elif kt == 7:
                                            nroot = rootp.tile(
                                                [P, 1024], bf16, tag="root"
                                            )
                                            nc.vector.tensor_add(
                                                nroot[:], acc_state.pop("g0")[:], acc[:]
                                            )
                                            acc_state["root"] = nroot
                                        else:
                                            nc.vector.tensor_add(
                                                root[:], root[:], acc[:]
                                            )
                                        if kt == 3:
                                            acc_state["g0"] = acc

                            scores(0)
                            scores(1)
                            for kt in range(16):
                                if kt < 14:
                                    scores(kt + 2)
                                e = ep.tile([P, 1024], bf16, tag="e")
                                nc.scalar.activation(
                                    e[:],
                                    pss_t.pop(kt)[:],
                                    EXP,
                                    bias=mask_sb[:, kt : kt + 1],
                                    scale=SCALE,
                                )
                                vsl = v_all[:, kt, h * HD : (h + 1) * HD]
                                nc.tensor.matmul(
                                    pso0[:],
                                    lhsT=vsl,
                                    rhs=e[:, 0:512],
                                    start=(kt == 0),
                                    stop=(kt == 15),
                                )
                                nc.tensor.matmul(
                                    pso1[:],
                                    lhsT=vsl,
                                    rhs=e[:, 512:1024],
                                    start=(kt == 0),
                                    stop=(kt == 15),
                                )
                                tree_push(e, kt)
                                if kt % 4 == 3:
                                    next(filler, None)
                            root = acc_state.pop("root")
                            # partition-sum + broadcast in ONE idle-Pool op;
                            # reciprocal in place, then normalize (both run
                            # well behind the PE, one q-chunk later).
                            den = bp.tile([P, 1024], bf16, tag="den")
                            nc.gpsimd.partition_all_reduce(
                                den[:], root[:], 128, bass_isa.ReduceOp.add
                            )
                            nc.vector.reciprocal(den[:], den[:])
                            nc.scalar.copy(attn_all[:, h, q0 : q0 + 512], pso0[:])
                            nc.scalar.copy(
                                attn_all[:, h, q0 + 512 : q0 + 1024], pso1[:]
                            )
                            nc.vector.tensor_mul(
                                attn_all[:, h, q0 : q0 + 1024],
                                attn_all[:, h, q0 : q0 + 1024],
                                den[:],
                            )

                    # ---- pipeline driver ----
                    import itertools

                    emit_w(0)
                    emit_w(1)
                    for _ in proj_chunks(0):
                        pass
                    leftover = iter(())
                    for h in range(H_PER_CORE):
                        if h + 2 < H_PER_CORE:
                            emit_w(h + 2)
                        if h == 5:
                            nc.sync.dma_start(out=wo0_sb[:], in_=wo[:, 0])
                        if h + 1 < H_PER_CORE:
                            gen = proj_chunks(h + 1)
                            if h == 6:
                                filler = itertools.islice(gen, 6)
                                leftover = gen
                            else:
                                filler = gen
                        else:
                            filler = leftover
                        attn_head(h, filler)
                        if h != 6:
                            for _ in filler:
                                pass

            # ---- stage 3: output projection, wo streamed in quarters ----
            with (
                tc.tile_pool(name="wo3", bufs=2) as wop,
                tc.tile_pool(name="os3", bufs=4) as osp,
                tc.tile_pool(name="ps3", bufs=4, space="PSUM") as ps3,
            ):
                for ec4 in range(4):
                    if ec4 == 0:
                        wo_sb = wo0_sb
                    else:
                        wo_sb = wop.tile([P, H_PER_CORE, 512], bf16, tag="wo")
                        nc.sync.dma_start(out=wo_sb[:], in_=wo[:, ec4])
                    for st in range(16):
                        ps = ps3.tile([P, 512], f32, tag="ps3")
                        for h in range(H_PER_CORE):
                            nc.tensor.matmul(
                                ps[:],
                                lhsT=attn_all[:, h, st * P : (st + 1) * P],
                                rhs=wo_sb[:, h],
                                start=(h == 0),
                                stop=(h == H_PER_CORE - 1),
                            )
                        o_sb = osp.tile([P, 512], bf16, tag="o3")
                        nc.vector.tensor_copy(o_sb[:], ps[:])
                        nc.sync.dma_start(
                            out=out_d[st * P : (st + 1) * P, ec4 * 512 : (ec4 + 1) * 512],
                            in_=o_sb[:],
                        )

    nc.compile()
    return nc


def _host_prep(inputs):
    from ml_dtypes import bfloat16

    x = np.asarray(inputs["x"], np.float32)
    fc = np.asarray(inputs["freqs_cos"], np.float32)
    fs = np.asarray(inputs["freqs_sin"], np.float32)
    mask = np.asarray(inputs["mask"], np.float32)
    wq = np.asarray(inputs["wq"], np.float32)
    wk = np.asarray(inputs["wk"], np.float32)
    wv = np.asarray(inputs["wv"], np.float32)
    wo = np.asarray(inputs["wo"], np.float32)

    def _img(a, nblk, w):
        # [R, nblk*w] -> [P, nblk, R//P, w]: one contiguous DMA per block
        R = a.shape[0]
        return np.ascontiguousarray(
            a.reshape(R // P, P, nblk, w).transpose(1, 2, 0, 3)
        ).astype(bfloat16)

    perm = np.concatenate([np.arange(0, HD, 2), np.arange(1, HD, 2)])
    cosT = np.ascontiguousarray(np.concatenate([fc.T, fc.T], 0)).astype(bfloat16)
    sinT = np.ascontiguousarray(np.concatenate([-fs.T, fs.T], 0)).astype(bfloat16)

    in_maps = []
    for c in range(NCORES):
        b, g = c // 2, c % 2
        colsel = np.concatenate([g * F + h * HD + perm for h in range(H_PER_CORE)])
        in_maps.append(
            {
                "xT": _img(x[b].T, 4, 512),
                "wq": _img(wq[:, colsel], H_PER_CORE, P),
                "wk": _img(wk[:, colsel], H_PER_CORE, P),
                "wv": _img(wv[:, g * F : (g + 1) * F], 2, 512),
                "wo": _img(wo[g * F : (g + 1) * F, :], 4, 512),
                "cosT": cosT,
                "sinT": sinT,
                "maskT": np.ascontiguousarray(mask[b].reshape(DT, P).T),
            }
        )
    return in_maps


def kernel(**inputs):
    from concourse.bass_utils import run_bass_kernel_spmd

    if "nc" not in _CACHE:
        _CACHE["nc"] = _build()
    nc = _CACHE["nc"]

    in_maps = _host_prep(inputs)
    res = run_bass_kernel_spmd(nc, in_maps, core_ids=list(range(NCORES)))
    out = np.empty((B, S, D), np.float32)
    for b in range(B):
        out[b] = res.results[2 * b]["out"].astype(np.float32) + res.results[
            2 * b + 1
        ]["out"].astype(np.float32)
    return out
